# revision 4
# baseline (speedup 1.0000x reference)
"""Two-layer GCN (PyG GCNConv x2 + log_softmax) on 8 Trainium2 NeuronCores.

Strategy (SPMD, one dispatch):
  - Nodes are padded to NPAD=100352 and sharded row-wise: core k owns nodes
    [k*12544, (k+1)*12544), i.e. 98 chunks of 128 nodes.
  - Layer tables: h = (x @ W) scaled per-row by dinv (dinv = 1/sqrt(deg+1)).
    Each core computes its slice, then an AllGather replicates the full table.
  - Edge aggregation per 128-node dst chunk: messages are gathered from the
    table by src index (dma_gather, 256B rows), a 0/1 one-hot matrix of local
    dst indices is built on DVE (iota is_equal), and PE matmul accumulates
      aggT[feat, node] += msgs[edge, feat].T @ onehot[edge, node]
    Self-loop term is one extra matmul with identity one-hot over the chunk's
    own (contiguous) table rows.  Post-scale by dinv[dst] restores the full
    sym-normalization: norm(e) = dinv[src]*dinv[dst].
  - dma_gather requires int16 indices, so the node table is processed in 4
    ranges of 25088 rows; each chunk's edges are bucketed by src range.
  - Bucket sizes are padded to a multiple of 128 uniformly over the 8 cores
    (SPMD: one program), pad slots gather row 0 of the range and have local
    dst -1 (zero one-hot column -> no contribution).
"""
import numpy as np

P = 128
NC = 8
CH = 98                 # chunks per core
SLICE = CH * P          # 12544 nodes per core
NPAD = NC * SLICE       # 100352
NGRP = 4
R = NPAD // NGRP        # 25088 table rows per gather range
N = 100000
E = 1600000
F_IN = 128
F_H = 64                # layer-1 width (table rows are 256B in f32)
F_OUT = 40


def _prep(x, edge_index, W1, b1, W2, b2):
    """Host-side graph preprocessing -> per-core input maps + structure."""
    src = np.asarray(edge_index[0], dtype=np.int64)
    dst = np.asarray(edge_index[1], dtype=np.int64)
    x = np.asarray(x, dtype=np.float32)
    W1 = np.asarray(W1, dtype=np.float32)
    W2 = np.asarray(W2, dtype=np.float32)
    b1 = np.asarray(b1, dtype=np.float32)
    b2 = np.asarray(b2, dtype=np.float32)

    n = x.shape[0]
    deg = np.bincount(dst, minlength=NPAD).astype(np.float32) + 1.0
    dinv = 1.0 / np.sqrt(deg)  # [NPAD]

    x_pad = np.zeros((NPAD, F_IN), np.float32)
    x_pad[:n] = x

    # ---- bucket edges by (core, chunk, group) ----
    core = dst // SLICE
    chunk = (dst % SLICE) // P
    grp = src // R
    counts = np.zeros((NC, CH, NGRP), np.int64)
    np.add.at(counts, (core, chunk, grp), 1)
    site_max = counts.max(axis=0)                      # [CH, NGRP]
    sizes = np.maximum(((site_max + 127) // 128) * 128, 128).astype(np.int64)

    # per-(chunk,group) slot/column offsets (shared by all cores)
    g_tiles = sizes // 128                             # [CH, NGRP] tiles per site
    chunk_tiles = g_tiles.sum(axis=1)                  # [CH]
    tile_off = np.zeros(CH + 1, np.int64)
    tile_off[1:] = np.cumsum(chunk_tiles)
    TT = int(tile_off[-1])                             # total gather tiles/core
    idx_col_off = np.zeros((CH, NGRP), np.int64)       # int16 col offsets
    acc = 0
    for c in range(CH):
        for g in range(NGRP):
            idx_col_off[c, g] = acc
            acc += sizes[c, g] // 16
    IDXCOLS = int(acc)

    # ---- per-core slot arrays ----
    order = np.lexsort((grp, chunk, core))             # stable bucket order
    so, do_ = src[order], dst[order]
    co, cho, go = core[order], chunk[order], grp[order]
    # rank of each edge within its (core,chunk,group) bucket
    bucket_id = (co * CH + cho) * NGRP + go
    uniq, first_pos = np.unique(bucket_id, return_index=True)
    start_of_bucket = np.zeros(NC * CH * NGRP, np.int64)
    start_of_bucket[uniq] = first_pos
    rank = np.arange(len(so)) - start_of_bucket[bucket_id]

    in_maps = []
    W2p = np.zeros((F_H, F_H), np.float32)
    W2p[:, :F_OUT] = W2
    b1c = b1.reshape(F_H, 1).astype(np.float32)
    b2c = np.zeros((F_H, 1), np.float32)
    b2c[:F_OUT, 0] = b2

    iota = np.broadcast_to(np.arange(P, dtype=np.float32), (P, P)).copy()
    ident = np.eye(P, dtype=np.float32)

    for k in range(NC):
        base = k * SLICE
        sel = co == k
        s_k, d_k = so[sel], do_[sel]
        ch_k, g_k, r_k = cho[sel], go[sel], rank[sel]

        idx16 = np.zeros((16, IDXCOLS), np.int16)
        dstloc = np.full((P, TT), -1.0, np.float32)

        slot_base = idx_col_off[ch_k, g_k] * 16        # slot index of bucket start
        j = slot_base + r_k                            # global slot (within idx16 flat)
        idx16[j % 16, j // 16] = (s_k - g_k * R).astype(np.int16)
        tile_idx = tile_off[ch_k] + (g_tiles[ch_k, :].cumsum(axis=1)[
            np.arange(len(ch_k)), g_k] - g_tiles[ch_k, g_k]) + r_k // 128
        dstloc[r_k % 128, tile_idx] = (d_k - base - ch_k * P).astype(np.float32)

        idx128 = np.tile(idx16, (8, 1))                # replicate across gpsimd cores

        xTs = np.ascontiguousarray(x_pad[base:base + SLICE].T)     # [128, SLICE]
        dinv_sl = dinv[base:base + SLICE]
        dinvb = np.broadcast_to(dinv_sl, (F_H, SLICE)).copy()      # [64, SLICE]
        dinvc = np.ascontiguousarray(dinv_sl.reshape(CH, P).T)     # [128, CH]

        in_maps.append({
            "xTs": xTs, "W1": W1, "W2p": W2p, "b1c": b1c, "b2c": b2c,
            "dinvb": dinvb, "dinvc": dinvc, "iota": iota, "ident": ident,
            "idx16": idx128, "dstloc": dstloc,
        })

    cfg = {
        "sizes": sizes, "g_tiles": g_tiles, "tile_off": tile_off,
        "idx_col_off": idx_col_off, "TT": TT, "IDXCOLS": IDXCOLS,
    }
    return cfg, in_maps


def _build(cfg):
    import concourse.bass as bass
    import concourse.mybir as mybir
    from concourse import bacc
    from concourse.tile import TileContext

    sizes = cfg["sizes"]
    g_tiles = cfg["g_tiles"]
    tile_off = cfg["tile_off"]
    idx_col_off = cfg["idx_col_off"]
    TT = cfg["TT"]
    IDXCOLS = cfg["IDXCOLS"]
    f32 = mybir.dt.float32

    nc = bacc.Bacc("TRN2", target_bir_lowering=False, debug=False, num_devices=NC)

    # inputs
    xTs = nc.dram_tensor("xTs", [P, SLICE], f32, kind="ExternalInput")
    W1 = nc.dram_tensor("W1", [F_IN, F_H], f32, kind="ExternalInput")
    W2p = nc.dram_tensor("W2p", [F_H, F_H], f32, kind="ExternalInput")
    b1c = nc.dram_tensor("b1c", [F_H, 1], f32, kind="ExternalInput")
    b2c = nc.dram_tensor("b2c", [F_H, 1], f32, kind="ExternalInput")
    dinvb = nc.dram_tensor("dinvb", [F_H, SLICE], f32, kind="ExternalInput")
    dinvc = nc.dram_tensor("dinvc", [P, CH], f32, kind="ExternalInput")
    iota = nc.dram_tensor("iota", [P, P], f32, kind="ExternalInput")
    ident = nc.dram_tensor("ident", [P, P], f32, kind="ExternalInput")
    idx16 = nc.dram_tensor("idx16", [P, IDXCOLS], mybir.dt.int16, kind="ExternalInput")
    dstloc = nc.dram_tensor("dstloc", [P, TT], f32, kind="ExternalInput")
    out = nc.dram_tensor("out", [SLICE, F_OUT], f32, kind="ExternalOutput")

    # internal DRAM
    t1s = nc.dram_tensor("t1s", [SLICE, F_H], f32, kind="Internal")
    ag1i = nc.dram_tensor("ag1i", [SLICE, F_H], f32, kind="Internal")
    t1f = nc.dram_tensor("t1f", [NPAD, F_H], f32, kind="Internal")
    t2s = nc.dram_tensor("t2s", [SLICE, F_H], f32, kind="Internal")
    ag2i = nc.dram_tensor("ag2i", [SLICE, F_H], f32, kind="Internal")
    t2f = nc.dram_tensor("t2f", [NPAD, F_H], f32, kind="Internal")

    TCMAX = int(g_tiles.sum(axis=1).max())
    groups = [list(range(NC))]

    with TileContext(nc) as tc:
        with (
            tc.tile_pool(name="const", bufs=1) as cp,
            tc.tile_pool(name="work", bufs=3) as wp,
            tc.tile_pool(name="msgs", bufs=2) as mp,
            tc.tile_pool(name="oh", bufs=4) as op_,
            tc.tile_pool(name="small", bufs=4) as sp,
            tc.tile_pool(name="psum", bufs=2, space="PSUM") as pp,
        ):
            # resident constants
            W1t = cp.tile([F_IN, F_H], f32)
            nc.sync.dma_start(out=W1t[:], in_=W1[:])
            W2t = cp.tile([F_H, F_H], f32)
            nc.sync.dma_start(out=W2t[:], in_=W2p[:])
            b1t = cp.tile([F_H, 1], f32)
            nc.sync.dma_start(out=b1t[:], in_=b1c[:])
            b2t = cp.tile([F_H, 1], f32)
            nc.sync.dma_start(out=b2t[:], in_=b2c[:])
            iot = cp.tile([P, P], f32)
            nc.sync.dma_start(out=iot[:], in_=iota[:])
            idt = cp.tile([P, P], f32)
            nc.sync.dma_start(out=idt[:], in_=ident[:])
            dvb = cp.tile([F_H, SLICE], f32)
            nc.sync.dma_start(out=dvb[:], in_=dinvb[:])
            dvc = cp.tile([P, CH], f32)
            nc.sync.dma_start(out=dvc[:], in_=dinvc[:])
            idxt = cp.tile([P, IDXCOLS], mybir.dt.int16)
            nc.sync.dma_start(out=idxt[:], in_=idx16[:])
            dlt = cp.tile([P, TT], f32)
            nc.sync.dma_start(out=dlt[:], in_=dstloc[:])

            size_regs = {}
            for sz in np.unique(sizes):
                size_regs[int(sz)] = nc.gpsimd.to_reg(int(sz))

            # ---- phase A: table1 slice = dinv * (x @ W1) ----
            for c in range(CH):
                xt = wp.tile([P, P], f32, tag="xt")
                nc.sync.dma_start(out=xt[:], in_=xTs[:, c * P:(c + 1) * P])
                h1p = pp.tile([P, F_H], f32, space="PSUM", tag="p64")
                nc.tensor.matmul(out=h1p[:], lhsT=xt[:], rhs=W1t[:],
                                 start=True, stop=True)
                t1t = wp.tile([P, F_H], f32, tag="t1t")
                nc.vector.tensor_scalar(out=t1t[:], in0=h1p[:],
                                        scalar1=dvc[:, c:c + 1], scalar2=None,
                                        op0=mybir.AluOpType.mult)
                nc.sync.dma_start(out=t1s[c * P:(c + 1) * P, :], in_=t1t[:])

            # ---- AllGather table1 ----
            nc.gpsimd.dma_start(out=ag1i[:], in_=t1s[:])
            nc.gpsimd.collective_compute(
                "AllGather", mybir.AluOpType.bypass, replica_groups=groups,
                ins=[ag1i[:]], outs=[t1f[:]],
            )

            def edge_layer(table_full, table_slice, layer):
                """Aggregate over edges for every chunk; returns nothing.

                layer==1: produce t2s rows (relu + W2 + dinv scaling).
                layer==2: produce final log_softmax rows into `out`.
                """
                for c in range(CH):
                    tc_tiles = int(g_tiles[c].sum())
                    msgs = mp.tile([P, TCMAX * F_H], f32, tag="msgs")
                    col = 0
                    for g in range(NGRP):
                        sz = int(sizes[c, g])
                        blocks = sz // 128
                        dst_ap = msgs[:, col * F_H:(col + blocks) * F_H]
                        nc.gpsimd.dma_gather(
                            out_ap=dst_ap.rearrange("p (t e) -> p t e", e=F_H),
                            in_ap=table_full[g * R:(g + 1) * R, :],
                            idxs_ap=idxt[:, int(idx_col_off[c, g]):
                                         int(idx_col_off[c, g]) + sz // 16],
                            num_idxs=sz,
                            num_idxs_reg=size_regs[sz],
                            elem_size=F_H,
                        )
                        col += blocks
                    agg = pp.tile([F_H, P], f32, space="PSUM", tag="agg")
                    for t in range(tc_tiles):
                        oh = op_.tile([P, P], f32, tag="oh")
                        nc.vector.tensor_scalar(
                            out=oh[:], in0=iot[:],
                            scalar1=dlt[:, int(tile_off[c]) + t:
                                        int(tile_off[c]) + t + 1],
                            scalar2=None, op0=mybir.AluOpType.is_equal)
                        nc.tensor.matmul(out=agg[:], lhsT=msgs[:, t * F_H:(t + 1) * F_H],
                                         rhs=oh[:], start=(t == 0), stop=False)
                    selfm = wp.tile([P, F_H], f32, tag="selfm")
                    nc.sync.dma_start(out=selfm[:],
                                      in_=table_slice[c * P:(c + 1) * P, :])
                    nc.tensor.matmul(out=agg[:], lhsT=selfm[:], rhs=idt[:],
                                     start=False, stop=True)
                    # scale columns by dinv[dst]
                    sc = wp.tile([F_H, P], f32, tag="sc")
                    nc.vector.tensor_tensor(out=sc[:], in0=agg[:],
                                            in1=dvb[:, c * P:(c + 1) * P],
                                            op=mybir.AluOpType.mult)
                    if layer == 1:
                        x2 = wp.tile([F_H, P], f32, tag="x2")
                        nc.scalar.activation(out=x2[:], in_=sc[:],
                                             func=mybir.ActivationFunctionType.Relu,
                                             bias=b1t[:, 0:1])
                        h2p = pp.tile([F_H, P], f32, space="PSUM", tag="h2p")
                        nc.tensor.matmul(out=h2p[:], lhsT=W2t[:], rhs=x2[:],
                                         start=True, stop=True)
                        hh = wp.tile([F_H, P], f32, tag="hh")
                        nc.vector.tensor_tensor(out=hh[:], in0=h2p[:],
                                                in1=dvb[:, c * P:(c + 1) * P],
                                                op=mybir.AluOpType.mult)
                        tp = pp.tile([P, F_H], f32, space="PSUM", tag="p64")
                        nc.tensor.transpose(out=tp[:], in_=hh[:], identity=idt[:F_H, :F_H])
                        t2t = wp.tile([P, F_H], f32, tag="t2t")
                        nc.scalar.activation(out=t2t[:], in_=tp[:],
                                             func=mybir.ActivationFunctionType.Copy)
                        nc.sync.dma_start(out=t2s[c * P:(c + 1) * P, :], in_=t2t[:])
                    else:
                        oo = wp.tile([F_H, P], f32, tag="oo")
                        nc.vector.tensor_scalar(out=oo[:], in0=sc[:],
                                                scalar1=b2t[:, 0:1], scalar2=None,
                                                op0=mybir.AluOpType.add)
                        tp2 = pp.tile([P, F_H], f32, space="PSUM", tag="p64")
                        nc.tensor.transpose(out=tp2[:], in_=oo[:], identity=idt[:F_H, :F_H])
                        # log_softmax over the first F_OUT columns
                        mx = sp.tile([P, 1], f32, tag="mx")
                        nc.vector.reduce_max(mx[:], tp2[:, :F_OUT],
                                             axis=mybir.AxisListType.X)
                        nmx = sp.tile([P, 1], f32, tag="nmx")
                        nc.vector.tensor_scalar(out=nmx[:], in0=mx[:],
                                                scalar1=-1.0, scalar2=None,
                                                op0=mybir.AluOpType.mult)
                        ex = wp.tile([P, F_OUT], f32, tag="ex")
                        sme = sp.tile([P, 1], f32, tag="sme")
                        nc.scalar.activation(out=ex[:], in_=tp2[:, :F_OUT],
                                             func=mybir.ActivationFunctionType.Exp,
                                             bias=nmx[:, 0:1], accum_out=sme[:])
                        ls = sp.tile([P, 1], f32, tag="ls")
                        nc.scalar.activation(out=ls[:], in_=sme[:],
                                             func=mybir.ActivationFunctionType.Ln)
                        ot = wp.tile([P, F_OUT], f32, tag="ot")
                        nc.vector.tensor_scalar(out=ot[:], in0=tp2[:, :F_OUT],
                                                scalar1=mx[:, 0:1],
                                                scalar2=ls[:, 0:1],
                                                op0=mybir.AluOpType.subtract,
                                                op1=mybir.AluOpType.subtract)
                        nc.sync.dma_start(out=out[c * P:(c + 1) * P, :], in_=ot[:])

            # ---- layer 1 edge aggregation -> table2 slice ----
            edge_layer(t1f, t1s, layer=1)

            # ---- AllGather table2 ----
            nc.gpsimd.dma_start(out=ag2i[:], in_=t2s[:])
            nc.gpsimd.collective_compute(
                "AllGather", mybir.AluOpType.bypass, replica_groups=groups,
                ins=[ag2i[:]], outs=[t2f[:]],
            )

            # ---- layer 2 edge aggregation -> log_softmax out ----
            edge_layer(t2f, t2s, layer=2)

    nc.finalize()
    return nc


_CACHE = {}


def kernel(x, edge_index, W1, b1, W2, b2):
    from concourse import bass_utils

    cfg, in_maps = _prep(x, edge_index, W1, b1, W2, b2)
    nc = _build(cfg)
    res = bass_utils.run_bass_kernel_spmd(
        nc, in_maps, core_ids=list(range(NC)), trace=False)
    full = np.concatenate([res.results[k]["out"] for k in range(NC)], axis=0)
    return np.ascontiguousarray(full[:N]).astype(np.float32)
